# revision 15
# baseline (speedup 1.0000x reference)
"""Trainium2 Bass kernel for nn_ButterflyFilter.

The reference applies, per length-512 row (flattened b*c*angles):
  zero-pad to 1024 -> 10-stage butterfly "FFT" -> elementwise filter
  (bit-reversed order) -> 10-stage butterfly "IFFT" -> real part of the
  first 512 entries.

The chain is linear in x, so it is one real 512x512 operator
W = Re(A)[:512, :512] with A the circulant filtered-convolution matrix.
W is an exactly symmetric Toeplitz matrix, W[o, i] = g[o - i], with g
the classic FBP ramp kernel: g[0] = 1/2, g[odd d] = -2/(pi d)^2,
g[even d] = 0. It decays like 1/d^2, so W is numerically BANDED: a
64-wide staircase band changes the result by ~1.6e-4 in relative norm;
bf16 operands and output store bring the total to ~2.6e-3 (measured),
still 7x under the 2e-2 gate.

Banded + Toeplitz lets each 128-row output chunk be computed from just
TWO 128-row input chunks taken on a 64-shifted grid:
  out[128o : 128o+128] = Ga @ c_o + Gb @ c_{o+1},
  c_j = x rows [128j - 64, 128j + 64)   (zero-padded at both ends)
with the SAME two 128x128 stationaries Ga, Gb for every o: 8 matmuls
per (b, c) tile, 16 per core (2 tiles/core on 8 cores), 64 KiB of
operator upload.

Schedule notes (raw Bass; everything learned from NTFF traces):
  - A DMA instruction costs ~0.7-0.8 us of descriptor-gen time on the
    issuing engine and per-queue bandwidth collapses with short
    partition lines, so inputs ship as 5 FUSED pieces with 1.5-3 KiB
    lines (one semaphore each - concurrently in-flight DMAs must not
    share a semaphore, their completions interleave).
  - Outputs stage in SBUF as one (128, 2048) bf16 tile per (b,c) tile
    and leave as 3 DMAs: tile0 whole (hidden under tile1 compute),
    tile1 in two halves as its accumulation groups close.
  - PSUM->SBUF copies are split column-wise between DVE (left half)
    and ACT (right half), halving the per-chunk copy latency and
    keeping either engine off the critical path.
  - A couple of warm-up matmuls on garbage SBUF right at program start
    keep the PE busy so the HAM clock ramp (1.2 -> 2.4 GHz, ~3.4 us of
    sustained activity) completes during the input stream.
"""

import os
import sys
import types
from contextlib import ExitStack

import numpy as np

import concourse.bass as bass
import concourse.mybir as mybir
from concourse.bass_utils import run_bass_kernel_spmd


def _ensure_axon_hooks():
    # concourse.bass_utils imports antenv.axon_hooks on the trace path; some
    # images lack that module. Provide a no-op holder so a BASS_TRACE env set
    # by the caller can't crash the run.
    try:
        import antenv.axon_hooks  # noqa: F401
    except Exception:
        m = types.ModuleType("antenv.axon_hooks")
        m._h = None
        m.set_axon_ntff_profile_hook = lambda h: setattr(m, "_h", h)
        m.get_axon_ntff_profile_hook = lambda: m._h
        sys.modules["antenv.axon_hooks"] = m


_ensure_axon_hooks()

N_CORES = 8
S = 512          # row length and angle count (moving dim)
NF = 1024        # padded length inside the reference
P = 128          # SBUF partitions / PE tile
KC = 5           # 64-shifted input chunks per tile
OC = 4           # output row chunks per tile
HALF = 64        # chunk-grid shift
BC_PER_CORE = 2
N_WARM = int(os.environ.get("BUTTERFLY_NWARM", "6"))

last_exec_time_ns = None
last_results = None


def _butterfly_np(tw, x, increasing):
    B, n = x.shape
    m = tw.shape[0]
    order = range(m) if increasing else range(m - 1, -1, -1)
    for idx in order:
        s = 1 << idx
        t = tw[idx].reshape(n // (2 * s), s, 2, 2)
        xr = x.reshape(B, n // (2 * s), 2, s)
        x = np.einsum('gjik,bgkj->bgij', t, xr).reshape(B, n)
    return x


def _compose_w(twiddle_fft, twiddle_ifft, fourier_filter_br):
    """Fold twiddles+filter into the dense operator W[o, i] (512x512 f64)."""
    tw_fft = np.asarray(twiddle_fft, dtype=np.float64)
    tw_ifft = np.asarray(twiddle_ifft, dtype=np.float64)
    filt = np.asarray(fourier_filter_br, dtype=np.float64)
    tf = tw_fft[0, ..., 0] + 1j * tw_fft[0, ..., 1]
    ti = tw_ifft[0, ..., 0] + 1j * tw_ifft[0, ..., 1]
    X = np.eye(NF, dtype=np.complex128)
    X = _butterfly_np(tf, X, increasing=False)
    X = X * filt[None, :]
    X = _butterfly_np(ti, X, increasing=True)
    return np.real(X[:S, :S]).T.copy()


def _band_stationaries(W):
    """lhsT operands: lhsT_a[i', o'] = g[o'-i'+64], lhsT_b = g[o'-i'-64]."""
    g = W[:, 0]  # g[|d|]; W is symmetric Toeplitz to ~3e-8
    D = np.arange(P)[None, :] - np.arange(P)[:, None]  # D[i', o'] = o' - i'
    return g[np.abs(D + HALF)], g[np.abs(D - HALF)]


def _shifted_chunks(x_bc, bf16):
    """(512, 512) tile -> (5, 128, 512) chunks on the 64-shifted grid."""
    xp = np.zeros((KC * P, S), dtype=bf16)
    xp[HALF:HALF + S] = x_bc.astype(bf16)
    return xp.reshape(KC, P, S)


def _build_nc():
    bf16 = mybir.dt.bfloat16
    f32 = mybir.dt.float32

    nc = bass.Bass()
    # Fused input pieces (per core). Sync queue: a0=[Ga|Gb|c0], a1=[c1|c2],
    # a2=[c3|c4] for tile0.  Scalar queue: b0=[c0|c1], b1=[c2|c3|c4] for
    # tile1.  All bf16.
    a0 = nc.declare_dram_parameter("a0", [P, 2 * P + S], bf16, isOutput=False)
    a1 = nc.declare_dram_parameter("a1", [P, 2 * S], bf16, isOutput=False)
    a2 = nc.declare_dram_parameter("a2", [P, 2 * S], bf16, isOutput=False)
    b0 = nc.declare_dram_parameter("b0", [P, 2 * S], bf16, isOutput=False)
    b1 = nc.declare_dram_parameter("b1", [P, 3 * S], bf16, isOutput=False)
    out0 = nc.declare_dram_parameter("out0", [P, OC * S], bf16, isOutput=True)
    out1 = nc.declare_dram_parameter("out1", [P, OC * S], bf16, isOutput=True)

    with ExitStack() as ctx:
        a0_sb = ctx.enter_context(nc.sbuf_tensor("a0_sb", [P, 2 * P + S], bf16))
        a1_sb = ctx.enter_context(nc.sbuf_tensor("a1_sb", [P, 2 * S], bf16))
        a2_sb = ctx.enter_context(nc.sbuf_tensor("a2_sb", [P, 2 * S], bf16))
        b0_sb = ctx.enter_context(nc.sbuf_tensor("b0_sb", [P, 2 * S], bf16))
        b1_sb = ctx.enter_context(nc.sbuf_tensor("b1_sb", [P, 3 * S], bf16))
        warm_sb = ctx.enter_context(nc.sbuf_tensor("warm_sb", [P, P + S], bf16))
        o_sb = [
            ctx.enter_context(nc.sbuf_tensor(f"o_sb{t}", [P, OC * S], bf16))
            for t in range(BC_PER_CORE)
        ]
        accs = [
            ctx.enter_context(nc.psum_tensor(f"acc{g}", [P, S], f32))
            for g in range(BC_PER_CORE * OC)
        ]
        s_a = [ctx.enter_context(nc.semaphore(f"s_a{i}")) for i in range(3)]
        s_b = [ctx.enter_context(nc.semaphore(f"s_b{i}")) for i in range(2)]
        s_pe = ctx.enter_context(nc.semaphore("s_pe"))
        s_cl = ctx.enter_context(nc.semaphore("s_cl"))   # DVE half-copies
        s_cr = ctx.enter_context(nc.semaphore("s_cr"))   # ACT half-copies
        s_out = ctx.enter_context(nc.semaphore("s_out"))
        block = ctx.enter_context(nc.Block())

        wa = a0_sb[:, 0:P]
        wb = a0_sb[:, P:2 * P]
        # tile0 chunks c0..c4 / tile1 chunks c0..c4
        c0 = [
            a0_sb[:, 2 * P:2 * P + S],
            a1_sb[:, bass.ts(0, S)], a1_sb[:, bass.ts(1, S)],
            a2_sb[:, bass.ts(0, S)], a2_sb[:, bass.ts(1, S)],
        ]
        c1 = [
            b0_sb[:, bass.ts(0, S)], b0_sb[:, bass.ts(1, S)],
            b1_sb[:, bass.ts(0, S)], b1_sb[:, bass.ts(1, S)],
            b1_sb[:, bass.ts(2, S)],
        ]

        @block.sync
        def _(sync):
            sync.dma_start(a0_sb[:], a0[:]).then_inc(s_a[0], 16)
            sync.dma_start(a1_sb[:], a1[:]).then_inc(s_a[1], 16)
            sync.dma_start(a2_sb[:], a2[:]).then_inc(s_a[2], 16)
            # tile1 first half [o0|o1] once copies g4, g5 are done
            if os.environ.get("BUTTERFLY_CAST", "tile_act") == "tile_act":
                sync.wait_ge(s_cr, 2)
            else:
                sync.wait_ge(s_cl, 6)
            sync.dma_start(out1[:, :2 * S], o_sb[1][:, :2 * S]).then_inc(s_out, 16)
            sync.wait_ge(s_out, 3 * 16)

        @block.tensor
        def _(tensor):
            for _ in range(N_WARM):
                nc.tensor.matmul(
                    accs[-1][:], warm_sb[:, :P], warm_sb[:, P:],
                    start=True, stop=True,
                )
            # tile0: mm order (piece gating in brackets)
            #   [a0] Ga0   [a1] Ga1 Gb0 Ga2 Gb1   [a2] Ga3 Gb2 Gb3
            def mm(acc, w_ap, c_ap, start, stop, t=None):
                m = nc.tensor.matmul(acc[:], w_ap, c_ap, start=start, stop=stop)
                if stop:
                    m.then_inc(s_pe, 1)

            tensor.wait_ge(s_a[0], 16)
            mm(accs[0], wa, c0[0], True, False)
            tensor.wait_ge(s_a[1], 16)
            mm(accs[1], wa, c0[1], True, False)
            mm(accs[0], wb, c0[1], False, True)
            mm(accs[2], wa, c0[2], True, False)
            mm(accs[1], wb, c0[2], False, True)
            tensor.wait_ge(s_a[2], 16)
            mm(accs[3], wa, c0[3], True, False)
            mm(accs[2], wb, c0[3], False, True)
            mm(accs[3], wb, c0[4], False, True)
            # tile1: [b0] Ga0 Ga1 Gb0   [b1] Ga2 Gb1 Ga3 Gb2 Gb3
            tensor.wait_ge(s_b[0], 16)
            mm(accs[4], wa, c1[0], True, False)
            mm(accs[5], wa, c1[1], True, False)
            mm(accs[4], wb, c1[1], False, True)
            tensor.wait_ge(s_b[1], 16)
            mm(accs[6], wa, c1[2], True, False)
            mm(accs[5], wb, c1[2], False, True)
            mm(accs[7], wa, c1[3], True, False)
            mm(accs[6], wb, c1[3], False, True)
            mm(accs[7], wb, c1[4], False, True)

        cast_mode = os.environ.get("BUTTERFLY_CAST", "tile_act")

        @block.vector
        def _(vector):
            # tile0 drains on DVE (and tile1 too in all-dve mode).
            n_dve = OC if cast_mode == "tile_act" else BC_PER_CORE * OC
            for g in range(n_dve):
                t, o = divmod(g, OC)
                vector.wait_ge(s_pe, g + 1)
                nc.vector.tensor_copy(
                    o_sb[t][:, bass.ts(o, S)], accs[g][:]
                ).then_inc(s_cl, 1)

        @block.scalar
        def _(scalar):
            scalar.dma_start(b0_sb[:], b0[:]).then_inc(s_b[0], 16)
            scalar.dma_start(b1_sb[:], b1[:]).then_inc(s_b[1], 16)
            scalar.wait_ge(s_cl, 4)
            scalar.dma_start(out0[:], o_sb[0][:]).then_inc(s_out, 16)
            if cast_mode == "tile_act":
                # tile1 drains on ACT (full-width activation copies).
                for g in range(OC, 2 * OC):
                    scalar.wait_ge(s_pe, g + 1)
                    nc.scalar.copy(
                        o_sb[1][:, bass.ts(g - OC, S)], accs[g][:]
                    ).then_inc(s_cr, 1)
            else:
                scalar.wait_ge(s_cl, 8)
                scalar.sem_inc(s_cr, 4)
            # tile1 second half [o2|o3]
            scalar.wait_ge(s_cr, 4)
            scalar.dma_start(
                out1[:, 2 * S:], o_sb[1][:, 2 * S:]
            ).then_inc(s_out, 16)

    return nc


def kernel(x, twiddle_fft, twiddle_ifft, fourier_filter_br):
    global last_exec_time_ns, last_results
    import ml_dtypes

    bf16 = ml_dtypes.bfloat16
    x = np.asarray(x, dtype=np.float32)
    b, c, s_len, a = x.shape
    assert (b, c, s_len, a) == (8, 2, S, S)

    W = _compose_w(twiddle_fft, twiddle_ifft, fourier_filter_br)
    la, lb = _band_stationaries(W)
    w_ab = np.concatenate([la, lb], axis=1).astype(bf16)  # (128, 256)
    x16 = x.reshape(b * c, S, S)

    in_maps = []
    for core in range(N_CORES):
        t0 = _shifted_chunks(x16[BC_PER_CORE * core], bf16)
        t1 = _shifted_chunks(x16[BC_PER_CORE * core + 1], bf16)
        cat = lambda parts: np.ascontiguousarray(np.concatenate(parts, axis=1))
        in_maps.append(
            {
                "a0": cat([w_ab, t0[0]]),
                "a1": cat([t0[1], t0[2]]),
                "a2": cat([t0[3], t0[4]]),
                "b0": cat([t1[0], t1[1]]),
                "b1": cat([t1[2], t1[3], t1[4]]),
            }
        )
    nc = _build_nc()
    trace = os.environ.get("BUTTERFLY_TRACE") == "1"
    res = run_bass_kernel_spmd(nc, in_maps, core_ids=list(range(N_CORES)), trace=trace)
    last_exec_time_ns = res.exec_time_ns
    last_results = res

    # outN[p, 512*o + a] = proj row 128*o + p of tile 2*core + N.
    q = np.empty((b * c, S, S), dtype=np.float32)
    for k in range(N_CORES):
        for t, name in enumerate(("out0", "out1")):
            y = np.asarray(res.results[k][name]).reshape(P, OC, S)
            q[BC_PER_CORE * k + t] = (
                y.transpose(1, 0, 2).reshape(S, S).astype(np.float32)
            )
    # q[bc, o, a] = proj.T[o, bc*512 + a]; reference output is
    # proj.T.reshape(b, c, s, a) — a pure reinterpret of the (512, 8192) buffer.
    out = q.transpose(1, 0, 2).reshape(S, b * c * a).reshape(b, c, s_len, a)
    return np.ascontiguousarray(out).astype(np.float32)


# revision 18
# speedup vs baseline: 1.0969x; 1.0969x over previous
"""Trainium2 Bass kernel for nn_ButterflyFilter.

The reference chain (pad -> butterfly FFT -> ramp filter in bit-reversed
order -> butterfly IFFT -> Re[:512]) is linear in x, so it is one real
512x512 operator W = Re(A)[:512, :512] with A circulant. W is an exactly
symmetric Toeplitz matrix W[o, i] = g[o - i] with g the FBP ramp kernel
(g[0] = 1/2, g[odd d] = -2/(pi d)^2, g[even d] = 0), which decays like
1/d^2: a 64-wide staircase band changes the result by ~1.6e-4 relative;
with bf16 operands and output store the total is ~2.6e-3 (measured),
7x under the 2e-2 gate.

Banded + Toeplitz => each 128-row output chunk needs TWO input chunks on
a 64-shifted grid:
  out[128o : 128o+128] = Ga @ c_o + Gb @ c_{o+1},
  c_j = x rows [128j - 64, 128j + 64)
with the same two 128x128 stationaries for every o. The edge chunks c_0
and c_4 have only 64 real rows, handled as K=64 matmuls against the
matching 64-row slices of Ga/Gb (no zero padding shipped): 8 matmuls
per (b, c) tile, 16 per core, 1.06 MiB in + 1 MiB out per core.

Schedule facts this implementation is built around (from NTFF traces):
  - DMA fabric is ~270 GB/s per core AGGREGATE across queues, so the
    two HWDGE queues are specialized: Sync carries every input piece in
    exactly PE consumption order; Scalar carries outputs. Competing
    input streams would starve the PE mid-stream.
  - Each dma_start costs ~0.7 us of descriptor generation on the
    issuing engine; pieces are fused to keep the count at 7 in / 3 out
    with 1-2 KiB partition lines.
  - Concurrently in-flight DMAs must not share a semaphore (their 16
    completion increments interleave), so every piece has its own.
  - PSUM->SBUF bf16 drains: DVE tensor_copy and ACT activation-copy
    both work full-width; copies are split 5/3 across the two engines
    so neither sits on the critical tail.
  - A few warm-up matmuls on garbage SBUF at program start keep the PE
    busy through the first DMA's ~1.5 us latency, so the HAM clock ramp
    (1.2 -> 2.4 GHz after ~3.4 us of sustained activity) completes
    while the stream is still input-paced.
"""

import os
import sys
import types
from contextlib import ExitStack

import numpy as np

import concourse.bass as bass
import concourse.mybir as mybir
from concourse.bass_utils import run_bass_kernel_spmd


def _ensure_axon_hooks():
    # concourse.bass_utils imports antenv.axon_hooks on the trace path; some
    # images lack that module. Provide a no-op holder so a BASS_TRACE env set
    # by the caller can't crash the run.
    try:
        import antenv.axon_hooks  # noqa: F401
    except Exception:
        m = types.ModuleType("antenv.axon_hooks")
        m._h = None
        m.set_axon_ntff_profile_hook = lambda h: setattr(m, "_h", h)
        m.get_axon_ntff_profile_hook = lambda: m._h
        sys.modules["antenv.axon_hooks"] = m


_ensure_axon_hooks()

N_CORES = 8
S = 512          # row length and angle count (moving dim)
NF = 1024        # padded length inside the reference
P = 128
H = 64           # chunk-grid shift / edge-chunk height
OC = 4           # output row chunks per tile
BC_PER_CORE = 2
N_WARM = int(os.environ.get("BUTTERFLY_NWARM", "3"))

last_exec_time_ns = None
last_results = None


def _butterfly_np(tw, x, increasing):
    B, n = x.shape
    m = tw.shape[0]
    order = range(m) if increasing else range(m - 1, -1, -1)
    for idx in order:
        s = 1 << idx
        t = tw[idx].reshape(n // (2 * s), s, 2, 2)
        xr = x.reshape(B, n // (2 * s), 2, s)
        x = np.einsum('gjik,bgkj->bgij', t, xr).reshape(B, n)
    return x


def _compose_w(twiddle_fft, twiddle_ifft, fourier_filter_br):
    """Fold twiddles+filter into the dense operator W[o, i] (512x512 f64)."""
    tw_fft = np.asarray(twiddle_fft, dtype=np.float64)
    tw_ifft = np.asarray(twiddle_ifft, dtype=np.float64)
    filt = np.asarray(fourier_filter_br, dtype=np.float64)
    tf = tw_fft[0, ..., 0] + 1j * tw_fft[0, ..., 1]
    ti = tw_ifft[0, ..., 0] + 1j * tw_ifft[0, ..., 1]
    X = np.eye(NF, dtype=np.complex128)
    X = _butterfly_np(tf, X, increasing=False)
    X = X * filt[None, :]
    X = _butterfly_np(ti, X, increasing=True)
    return np.real(X[:S, :S]).T.copy()


def _band_stationaries(W):
    """lhsT operands: lhsT_a[i', o'] = g[o'-i'+64], lhsT_b = g[o'-i'-64]."""
    g = W[:, 0]  # g[|d|]; W is symmetric Toeplitz to ~3e-8
    D = np.arange(P)[None, :] - np.arange(P)[:, None]  # D[i', o'] = o' - i'
    return g[np.abs(D + H)], g[np.abs(D - H)]


def _build_nc():
    bf16 = mybir.dt.bfloat16
    f32 = mybir.dt.float32

    nc = bass.Bass()
    # Input pieces, declared in Sync-queue (= PE consumption) order:
    #   i0 = (128, 512)  [Ga | Gb | edge]: edge rows 0-63 = [Ga_lo | Gb_hi]
    #   per tile: (64, 1024) [c0' | c4'], (128, 1024) [c1 | c2], (128, 512) c3
    i0 = nc.declare_dram_parameter("i0", [P, 4 * P], bf16, isOutput=False)
    xe = [
        nc.declare_dram_parameter(f"xe{t}", [H, 2 * S], bf16, isOutput=False)
        for t in range(BC_PER_CORE)
    ]
    x12 = [
        nc.declare_dram_parameter(f"x12_{t}", [P, 2 * S], bf16, isOutput=False)
        for t in range(BC_PER_CORE)
    ]
    x3 = [
        nc.declare_dram_parameter(f"x3_{t}", [P, S], bf16, isOutput=False)
        for t in range(BC_PER_CORE)
    ]
    out0 = nc.declare_dram_parameter("out0", [P, OC * S], bf16, isOutput=True)
    out1 = nc.declare_dram_parameter("out1", [P, OC * S], bf16, isOutput=True)

    with ExitStack() as ctx:
        w_sb = ctx.enter_context(nc.sbuf_tensor("w_sb", [P, 4 * P], bf16))
        xe_sb = [
            ctx.enter_context(nc.sbuf_tensor(f"xe_sb{t}", [H, 2 * S], bf16))
            for t in range(BC_PER_CORE)
        ]
        x12_sb = [
            ctx.enter_context(nc.sbuf_tensor(f"x12_sb{t}", [P, 2 * S], bf16))
            for t in range(BC_PER_CORE)
        ]
        x3_sb = [
            ctx.enter_context(nc.sbuf_tensor(f"x3_sb{t}", [P, S], bf16))
            for t in range(BC_PER_CORE)
        ]
        warm_sb = ctx.enter_context(nc.sbuf_tensor("warm_sb", [P, P + S], bf16))
        o_sb = [
            ctx.enter_context(nc.sbuf_tensor(f"o_sb{t}", [P, OC * S], bf16))
            for t in range(BC_PER_CORE)
        ]
        accs = [
            ctx.enter_context(nc.psum_tensor(f"acc{g}", [P, S], f32))
            for g in range(BC_PER_CORE * OC)
        ]
        s_i = [ctx.enter_context(nc.semaphore(f"s_i{j}")) for j in range(7)]
        s_pe = ctx.enter_context(nc.semaphore("s_pe"))
        s_cl = ctx.enter_context(nc.semaphore("s_cl"))   # DVE copies
        s_cr = ctx.enter_context(nc.semaphore("s_cr"))   # ACT copies
        s_out = ctx.enter_context(nc.semaphore("s_out"))
        block = ctx.enter_context(nc.Block())

        ga = w_sb[:, 0:P]
        gb = w_sb[:, P:2 * P]
        ga_lo = w_sb[0:H, 2 * P:3 * P]   # rows 64-127 of Ga
        gb_hi = w_sb[0:H, 3 * P:4 * P]   # rows 0-63 of Gb
        # Per tile: c0' (64 real rows), c1, c2, c3, c4' (64 rows)
        c0p = [xe_sb[t][:, 0:S] for t in range(2)]
        c4p = [xe_sb[t][:, S:2 * S] for t in range(2)]
        c1 = [x12_sb[t][:, 0:S] for t in range(2)]
        c2 = [x12_sb[t][:, S:2 * S] for t in range(2)]
        c3 = [x3_sb[t][:] for t in range(2)]

        @block.sync
        def _(sync):
            sync.dma_start(w_sb[:], i0[:]).then_inc(s_i[0], 16)
            for t in range(BC_PER_CORE):
                sync.dma_start(xe_sb[t][:], xe[t][:]).then_inc(s_i[3 * t + 1], 16)
                sync.dma_start(x12_sb[t][:], x12[t][:]).then_inc(s_i[3 * t + 2], 16)
                sync.dma_start(x3_sb[t][:], x3[t][:]).then_inc(s_i[3 * t + 3], 16)
            # tile1 [o0|o1] after copies g4 (DVE #4) and g5 (ACT #2)
            sync.wait_ge(s_cl, 4)
            sync.wait_ge(s_cr, 2)
            sync.dma_start(out1[:, :2 * S], o_sb[1][:, :2 * S]).then_inc(s_out, 16)
            sync.wait_ge(s_out, 3 * 16)

        @block.tensor
        def _(tensor):
            for _ in range(N_WARM):
                nc.tensor.matmul(
                    accs[-1][:], warm_sb[:, :P], warm_sb[:, P:],
                    start=True, stop=True,
                )
            tensor.wait_ge(s_i[0], 16)
            for t in range(BC_PER_CORE):
                a = OC * t

                def mm(g, w_ap, c_ap, start, stop):
                    m = nc.tensor.matmul(
                        accs[g][:], w_ap, c_ap, start=start, stop=stop
                    )
                    if stop:
                        m.then_inc(s_pe, 1)

                tensor.wait_ge(s_i[3 * t + 1], 16)
                mm(a + 0, ga_lo, c0p[t], True, False)
                tensor.wait_ge(s_i[3 * t + 2], 16)
                mm(a + 1, ga, c1[t], True, False)
                mm(a + 0, gb, c1[t], False, True)
                mm(a + 2, ga, c2[t], True, False)
                mm(a + 1, gb, c2[t], False, True)
                tensor.wait_ge(s_i[3 * t + 3], 16)
                mm(a + 3, ga, c3[t], True, False)
                mm(a + 2, gb, c3[t], False, True)
                mm(a + 3, gb_hi, c4p[t], False, True)

        @block.vector
        def _(vector):
            # DVE drains groups 0,1,2 (tile0) and 4,6 (tile1).
            for g, thr in ((0, 1), (1, 2), (2, 3), (4, 5), (6, 7)):
                t, o = divmod(g, OC)
                vector.wait_ge(s_pe, thr)
                nc.vector.tensor_copy(o_sb[t][:, bass.ts(o, S)], accs[g][:])
                # Signal on a drain so the SBUF write is visible to the DMA
                # engines before the consumer queue fires (pipe.py idiom).
                vector.drain().then_inc(s_cl, 1)

        @block.scalar
        def _(scalar):
            # ACT drains groups 3 (tile0) and 5, 7 (tile1), and issues all
            # remaining output DMAs on its queue.
            scalar.wait_ge(s_pe, 4)
            nc.scalar.copy(o_sb[0][:, bass.ts(3, S)], accs[3][:])
            scalar.drain().then_inc(s_cr, 1)
            scalar.wait_ge(s_cl, 3)
            scalar.dma_start(out0[:], o_sb[0][:]).then_inc(s_out, 16)
            scalar.wait_ge(s_pe, 6)
            nc.scalar.copy(o_sb[1][:, bass.ts(1, S)], accs[5][:])
            scalar.drain().then_inc(s_cr, 1)
            scalar.wait_ge(s_pe, 8)
            nc.scalar.copy(o_sb[1][:, bass.ts(3, S)], accs[7][:])
            scalar.drain().then_inc(s_cr, 1)
            # tile1 [o2|o3] after copies g6 (DVE #5) and g7 (ACT #3)
            scalar.wait_ge(s_cl, 5)
            scalar.dma_start(
                out1[:, 2 * S:], o_sb[1][:, 2 * S:]
            ).then_inc(s_out, 16)

    return nc


def kernel(x, twiddle_fft, twiddle_ifft, fourier_filter_br):
    global last_exec_time_ns, last_results
    import ml_dtypes

    bf16 = ml_dtypes.bfloat16
    x = np.asarray(x, dtype=np.float32)
    b, c, s_len, a = x.shape
    assert (b, c, s_len, a) == (8, 2, S, S)

    W = _compose_w(twiddle_fft, twiddle_ifft, fourier_filter_br)
    la, lb = _band_stationaries(W)
    edge = np.zeros((P, 2 * P))
    edge[0:H, 0:P] = la[H:, :]     # Ga rows 64-127, for c0'
    edge[0:H, P:2 * P] = lb[:H, :]  # Gb rows 0-63, for c4'
    i0 = np.ascontiguousarray(
        np.concatenate([la, lb, edge], axis=1).astype(bf16)
    )
    x16 = x.reshape(b * c, S, S)

    in_maps = []
    for core in range(N_CORES):
        m = {"i0": i0}
        for t in range(BC_PER_CORE):
            xb = x16[BC_PER_CORE * core + t].astype(bf16)
            m[f"xe{t}"] = np.ascontiguousarray(
                np.concatenate([xb[0:H], xb[S - H:S]], axis=1)
            )
            m[f"x12_{t}"] = np.ascontiguousarray(
                np.concatenate([xb[H:H + P], xb[H + P:H + 2 * P]], axis=1)
            )
            m[f"x3_{t}"] = np.ascontiguousarray(xb[H + 2 * P:H + 3 * P])
        in_maps.append(m)
    nc = _build_nc()
    trace = os.environ.get("BUTTERFLY_TRACE") == "1"
    res = run_bass_kernel_spmd(nc, in_maps, core_ids=list(range(N_CORES)), trace=trace)
    last_exec_time_ns = res.exec_time_ns
    last_results = res

    # outN[p, 512*o + a] = proj row 128*o + p of tile 2*core + N.
    q = np.empty((b * c, S, S), dtype=np.float32)
    for k in range(N_CORES):
        for t, name in enumerate(("out0", "out1")):
            y = np.asarray(res.results[k][name]).reshape(P, OC, S)
            q[BC_PER_CORE * k + t] = (
                y.transpose(1, 0, 2).reshape(S, S).astype(np.float32)
            )
    # q[bc, o, a] = proj.T[o, bc*512 + a]; reference output is
    # proj.T.reshape(b, c, s, a) — a pure reinterpret of the (512, 8192) buffer.
    out = q.transpose(1, 0, 2).reshape(S, b * c * a).reshape(b, c, s_len, a)
    return np.ascontiguousarray(out).astype(np.float32)


# revision 19
# speedup vs baseline: 1.1306x; 1.0307x over previous
"""Trainium2 Bass kernel for nn_ButterflyFilter.

The reference chain (pad -> butterfly FFT -> ramp filter in bit-reversed
order -> butterfly IFFT -> Re[:512]) is linear in x, so it is one real
512x512 operator W = Re(A)[:512, :512] with A circulant. W is an exactly
symmetric Toeplitz matrix W[o, i] = g[o - i] with g the FBP ramp kernel
(g[0] = 1/2, g[odd d] = -2/(pi d)^2, g[even d] = 0), which decays like
1/d^2: a 64-wide staircase band changes the result by ~1.6e-4 relative;
with bf16 operands and output store the total is ~2.6e-3 (measured),
7x under the 2e-2 gate.

Banded + Toeplitz => each 128-row output chunk needs TWO input chunks on
a 64-shifted grid:
  out[128o : 128o+128] = Ga @ c_o + Gb @ c_{o+1},
  c_j = x rows [128j - 64, 128j + 64)   (zero-padded at the ends)
with the same two 128x128 stationaries for every o: 8 matmuls per
(b, c) tile, 16 per core (2 tiles/core, 8 cores), 64 KiB of operator.

Schedule facts this implementation is built around (from NTFF traces):
  - The DMA fabric is ~270 GB/s per core AGGREGATE across queues, so
    queues are specialized: Sync carries the 5 input pieces in exact PE
    consumption order; Scalar's queue carries outputs. Competing input
    streams starve the PE mid-stream.
  - A dma_start costs ~0.6-0.7 us of descriptor-gen on the issuing
    engine: inputs are fused into 5 pieces (64K + 256K/384K per tile)
    with 1-3 KiB partition lines.
  - Concurrently in-flight DMAs must not share a semaphore (their 16
    completion increments interleave out of order): one per piece.
  - PSUM->SBUF bf16 drains: DVE tensor_copy signals with then_inc
    directly (proven safe); ACT activation-copies are kept OFF the
    critical tail and signal via an explicit pipeline drain.
  - ~6 warm-up matmuls on garbage SBUF bridge program start to the
    first piece's arrival so the HAM clock ramp (1.2 -> 2.4 GHz after
    ~3.4 us of sustained PE activity) completes before the real stream;
    any PE idle gap resets the ramp credit.
"""

import os
import sys
import types
from contextlib import ExitStack

import numpy as np

import concourse.bass as bass
import concourse.mybir as mybir
from concourse.bass_utils import run_bass_kernel_spmd


def _ensure_axon_hooks():
    # concourse.bass_utils imports antenv.axon_hooks on the trace path; some
    # images lack that module. Provide a no-op holder so a BASS_TRACE env set
    # by the caller can't crash the run.
    try:
        import antenv.axon_hooks  # noqa: F401
    except Exception:
        m = types.ModuleType("antenv.axon_hooks")
        m._h = None
        m.set_axon_ntff_profile_hook = lambda h: setattr(m, "_h", h)
        m.get_axon_ntff_profile_hook = lambda: m._h
        sys.modules["antenv.axon_hooks"] = m


_ensure_axon_hooks()

N_CORES = 8
S = 512          # row length and angle count (moving dim)
NF = 1024        # padded length inside the reference
P = 128
H = 64           # chunk-grid shift
OC = 4           # output row chunks per tile
BC_PER_CORE = 2
N_WARM = int(os.environ.get("BUTTERFLY_NWARM", "6"))

last_exec_time_ns = None
last_results = None


def _butterfly_np(tw, x, increasing):
    B, n = x.shape
    m = tw.shape[0]
    order = range(m) if increasing else range(m - 1, -1, -1)
    for idx in order:
        s = 1 << idx
        t = tw[idx].reshape(n // (2 * s), s, 2, 2)
        xr = x.reshape(B, n // (2 * s), 2, s)
        x = np.einsum('gjik,bgkj->bgij', t, xr).reshape(B, n)
    return x


def _compose_w(twiddle_fft, twiddle_ifft, fourier_filter_br):
    """Fold twiddles+filter into the dense operator W[o, i] (512x512 f64)."""
    tw_fft = np.asarray(twiddle_fft, dtype=np.float64)
    tw_ifft = np.asarray(twiddle_ifft, dtype=np.float64)
    filt = np.asarray(fourier_filter_br, dtype=np.float64)
    tf = tw_fft[0, ..., 0] + 1j * tw_fft[0, ..., 1]
    ti = tw_ifft[0, ..., 0] + 1j * tw_ifft[0, ..., 1]
    X = np.eye(NF, dtype=np.complex128)
    X = _butterfly_np(tf, X, increasing=False)
    X = X * filt[None, :]
    X = _butterfly_np(ti, X, increasing=True)
    return np.real(X[:S, :S]).T.copy()


def _band_stationaries(W):
    """lhsT operands: lhsT_a[i', o'] = g[o'-i'+64], lhsT_b = g[o'-i'-64]."""
    g = W[:, 0]  # g[|d|]; W is symmetric Toeplitz to ~3e-8
    D = np.arange(P)[None, :] - np.arange(P)[:, None]  # D[i', o'] = o' - i'
    return g[np.abs(D + H)], g[np.abs(D - H)]


def _build_nc():
    bf16 = mybir.dt.bfloat16
    f32 = mybir.dt.float32

    nc = bass.Bass()
    # Input pieces in Sync-queue (= PE consumption) order:
    #   w = (128, 256) [Ga | Gb]
    #   per tile: pa = (128, 1024) [c0 | c1], pb = (128, 1536) [c2 | c3 | c4]
    w = nc.declare_dram_parameter("w", [P, 2 * P], bf16, isOutput=False)
    pa = [
        nc.declare_dram_parameter(f"pa{t}", [P, 2 * S], bf16, isOutput=False)
        for t in range(BC_PER_CORE)
    ]
    pb = [
        nc.declare_dram_parameter(f"pb{t}", [P, 3 * S], bf16, isOutput=False)
        for t in range(BC_PER_CORE)
    ]
    out0 = nc.declare_dram_parameter("out0", [P, OC * S], bf16, isOutput=True)
    out1 = nc.declare_dram_parameter("out1", [P, OC * S], bf16, isOutput=True)

    with ExitStack() as ctx:
        w_sb = ctx.enter_context(nc.sbuf_tensor("w_sb", [P, 2 * P], bf16))
        pa_sb = [
            ctx.enter_context(nc.sbuf_tensor(f"pa_sb{t}", [P, 2 * S], bf16))
            for t in range(BC_PER_CORE)
        ]
        pb_sb = [
            ctx.enter_context(nc.sbuf_tensor(f"pb_sb{t}", [P, 3 * S], bf16))
            for t in range(BC_PER_CORE)
        ]
        warm_sb = ctx.enter_context(nc.sbuf_tensor("warm_sb", [P, P + S], bf16))
        o_sb = [
            ctx.enter_context(nc.sbuf_tensor(f"o_sb{t}", [P, OC * S], bf16))
            for t in range(BC_PER_CORE)
        ]
        accs = [
            ctx.enter_context(nc.psum_tensor(f"acc{g}", [P, S], f32))
            for g in range(BC_PER_CORE * OC)
        ]
        s_i = [ctx.enter_context(nc.semaphore(f"s_i{j}")) for j in range(5)]
        s_pe = ctx.enter_context(nc.semaphore("s_pe"))
        s_cl = ctx.enter_context(nc.semaphore("s_cl"))   # DVE copies
        s_cr = ctx.enter_context(nc.semaphore("s_cr"))   # ACT copies
        s_out = ctx.enter_context(nc.semaphore("s_out"))
        block = ctx.enter_context(nc.Block())

        ga = w_sb[:, 0:P]
        gb = w_sb[:, P:2 * P]
        cs = [
            [
                pa_sb[t][:, 0:S], pa_sb[t][:, S:2 * S],
                pb_sb[t][:, 0:S], pb_sb[t][:, S:2 * S], pb_sb[t][:, 2 * S:],
            ]
            for t in range(BC_PER_CORE)
        ]

        @block.sync
        def _(sync):
            sync.dma_start(w_sb[:], w[:]).then_inc(s_i[0], 16)
            for t in range(BC_PER_CORE):
                sync.dma_start(pa_sb[t][:], pa[t][:]).then_inc(s_i[2 * t + 1], 16)
                sync.dma_start(pb_sb[t][:], pb[t][:]).then_inc(s_i[2 * t + 2], 16)
            # tile1 [o0|o1] after copies g4 (DVE #4) and g5 (ACT #2)
            sync.wait_ge(s_cl, 4)
            sync.wait_ge(s_cr, 2)
            sync.dma_start(out1[:, :2 * S], o_sb[1][:, :2 * S]).then_inc(s_out, 16)
            sync.wait_ge(s_out, 3 * 16)

        @block.tensor
        def _(tensor):
            for _ in range(N_WARM):
                nc.tensor.matmul(
                    accs[-1][:], warm_sb[:, :P], warm_sb[:, P:],
                    start=True, stop=True,
                )
            tensor.wait_ge(s_i[0], 16)
            for t in range(BC_PER_CORE):
                a = OC * t
                c = cs[t]

                def mm(g, w_ap, c_ap, start, stop):
                    m = nc.tensor.matmul(
                        accs[g][:], w_ap, c_ap, start=start, stop=stop
                    )
                    if stop:
                        m.then_inc(s_pe, 1)

                tensor.wait_ge(s_i[2 * t + 1], 16)
                mm(a + 0, ga, c[0], True, False)
                mm(a + 1, ga, c[1], True, False)
                mm(a + 0, gb, c[1], False, True)
                tensor.wait_ge(s_i[2 * t + 2], 16)
                mm(a + 2, ga, c[2], True, False)
                mm(a + 1, gb, c[2], False, True)
                mm(a + 3, ga, c[3], True, False)
                mm(a + 2, gb, c[3], False, True)
                mm(a + 3, gb, c[4], False, True)

        @block.vector
        def _(vector):
            # DVE drains groups 0,1,2 (tile0) and 4,6,7 (tile1); then_inc
            # rides the copy itself (v3.1-proven safe for DVE).
            for g, thr in ((0, 1), (1, 2), (2, 3), (4, 5), (6, 7), (7, 8)):
                t, o = divmod(g, OC)
                vector.wait_ge(s_pe, thr)
                nc.vector.tensor_copy(
                    o_sb[t][:, bass.ts(o, S)], accs[g][:]
                ).then_inc(s_cl, 1)

        @block.scalar
        def _(scalar):
            # ACT drains the two mid-stream groups 3 and 5 (signalled via an
            # explicit pipeline drain for DMA-read visibility) and issues the
            # out0 / out1-second-half DMAs on its queue.
            scalar.wait_ge(s_pe, 4)
            nc.scalar.copy(o_sb[0][:, bass.ts(3, S)], accs[3][:])
            scalar.drain().then_inc(s_cr, 1)
            scalar.wait_ge(s_cl, 3)
            scalar.dma_start(out0[:], o_sb[0][:]).then_inc(s_out, 16)
            scalar.wait_ge(s_pe, 6)
            nc.scalar.copy(o_sb[1][:, bass.ts(1, S)], accs[5][:])
            scalar.drain().then_inc(s_cr, 1)
            # tile1 [o2|o3] after DVE copies g6 (#5) and g7 (#6)
            scalar.wait_ge(s_cl, 6)
            scalar.dma_start(
                out1[:, 2 * S:], o_sb[1][:, 2 * S:]
            ).then_inc(s_out, 16)

    return nc


def kernel(x, twiddle_fft, twiddle_ifft, fourier_filter_br):
    global last_exec_time_ns, last_results
    import ml_dtypes

    bf16 = ml_dtypes.bfloat16
    x = np.asarray(x, dtype=np.float32)
    b, c, s_len, a = x.shape
    assert (b, c, s_len, a) == (8, 2, S, S)

    W = _compose_w(twiddle_fft, twiddle_ifft, fourier_filter_br)
    la, lb = _band_stationaries(W)
    w_piece = np.ascontiguousarray(
        np.concatenate([la, lb], axis=1).astype(bf16)
    )
    x16 = x.reshape(b * c, S, S)
    zpad = np.zeros((H, S), dtype=bf16)

    in_maps = []
    for core in range(N_CORES):
        m = {"w": w_piece}
        for t in range(BC_PER_CORE):
            xb = x16[BC_PER_CORE * core + t].astype(bf16)
            c0 = np.concatenate([zpad, xb[0:H]], axis=0)
            c1 = xb[H:H + P]
            c2 = xb[H + P:H + 2 * P]
            c3 = xb[H + 2 * P:H + 3 * P]
            c4 = np.concatenate([xb[H + 3 * P:], zpad], axis=0)
            m[f"pa{t}"] = np.ascontiguousarray(np.concatenate([c0, c1], axis=1))
            m[f"pb{t}"] = np.ascontiguousarray(
                np.concatenate([c2, c3, c4], axis=1)
            )
        in_maps.append(m)
    nc = _build_nc()
    trace = os.environ.get("BUTTERFLY_TRACE") == "1"
    res = run_bass_kernel_spmd(nc, in_maps, core_ids=list(range(N_CORES)), trace=trace)
    last_exec_time_ns = res.exec_time_ns
    last_results = res

    # outN[p, 512*o + a] = proj row 128*o + p of tile 2*core + N.
    q = np.empty((b * c, S, S), dtype=np.float32)
    for k in range(N_CORES):
        for t, name in enumerate(("out0", "out1")):
            y = np.asarray(res.results[k][name]).reshape(P, OC, S)
            q[BC_PER_CORE * k + t] = (
                y.transpose(1, 0, 2).reshape(S, S).astype(np.float32)
            )
    # q[bc, o, a] = proj.T[o, bc*512 + a]; reference output is
    # proj.T.reshape(b, c, s, a) — a pure reinterpret of the (512, 8192) buffer.
    out = q.transpose(1, 0, 2).reshape(S, b * c * a).reshape(b, c, s_len, a)
    return np.ascontiguousarray(out).astype(np.float32)


# revision 24
# speedup vs baseline: 1.1771x; 1.0411x over previous
"""Trainium2 Bass kernel for nn_ButterflyFilter.

The reference chain (pad -> butterfly FFT -> ramp filter in bit-reversed
order -> butterfly IFFT -> Re[:512]) is linear in x, so it is one real
512x512 operator W = Re(A)[:512, :512] with A circulant. W is an exactly
symmetric Toeplitz matrix W[o, i] = g[o - i] with g the FBP ramp kernel
(g[0] = 1/2, g[odd d] = -2/(pi d)^2, g[even d] = 0), which decays like
1/d^2: a 64-wide staircase band changes the result by ~1.6e-4 relative;
with bf16 operands and output store the total is ~2.6e-3 (measured),
7x under the 2e-2 gate.

Banded + Toeplitz => each 128-row output chunk needs TWO input chunks on
a 64-shifted grid:
  out[128o : 128o+128] = Ga @ c_o + Gb @ c_{o+1},
  c_j = x rows [128j - 64, 128j + 64)   (zero-padded at the ends)
with the same two 128x128 stationaries for every o: 8 matmuls per
(b, c) tile, 16 per core (2 tiles/core, 8 cores), 64 KiB of operator.

Schedule facts this implementation is built around (from NTFF traces):
  - The DMA fabric is ~270 GB/s per core AGGREGATE across queues, so
    queues are specialized: Sync carries the 5 input pieces in exact PE
    consumption order; Scalar's queue carries outputs. Competing input
    streams starve the PE mid-stream.
  - A dma_start costs ~0.6-0.7 us of descriptor-gen on the issuing
    engine: inputs are fused into 5 pieces (64K + 256K/384K per tile)
    with 1-3 KiB partition lines.
  - Concurrently in-flight DMAs must not share a semaphore (their 16
    completion increments interleave out of order): one per piece.
  - PSUM->SBUF bf16 drains: DVE tensor_copy signals with then_inc
    directly (proven safe); ACT activation-copies are kept OFF the
    critical tail and signal via an explicit pipeline drain.
  - ~6 warm-up matmuls on garbage SBUF bridge program start to the
    first piece's arrival so the HAM clock ramp (1.2 -> 2.4 GHz after
    ~3.4 us of sustained PE activity) completes before the real stream;
    any PE idle gap resets the ramp credit.
"""

import os
import sys
import types
from contextlib import ExitStack

import numpy as np

import concourse.bass as bass
import concourse.mybir as mybir
from concourse.bass_utils import run_bass_kernel_spmd


def _ensure_axon_hooks():
    # concourse.bass_utils imports antenv.axon_hooks on the trace path; some
    # images lack that module. Provide a no-op holder so a BASS_TRACE env set
    # by the caller can't crash the run.
    try:
        import antenv.axon_hooks  # noqa: F401
    except Exception:
        m = types.ModuleType("antenv.axon_hooks")
        m._h = None
        m.set_axon_ntff_profile_hook = lambda h: setattr(m, "_h", h)
        m.get_axon_ntff_profile_hook = lambda: m._h
        sys.modules["antenv.axon_hooks"] = m


_ensure_axon_hooks()

N_CORES = 8
S = 512          # row length and angle count (moving dim)
NF = 1024        # padded length inside the reference
P = 128
H = 64           # chunk-grid shift
OC = 4           # output row chunks per tile
BC_PER_CORE = 2
N_WARM = int(os.environ.get("BUTTERFLY_NWARM", "22"))

last_exec_time_ns = None
last_results = None


def _butterfly_np(tw, x, increasing):
    B, n = x.shape
    m = tw.shape[0]
    order = range(m) if increasing else range(m - 1, -1, -1)
    for idx in order:
        s = 1 << idx
        t = tw[idx].reshape(n // (2 * s), s, 2, 2)
        xr = x.reshape(B, n // (2 * s), 2, s)
        x = np.einsum('gjik,bgkj->bgij', t, xr).reshape(B, n)
    return x


def _compose_w(twiddle_fft, twiddle_ifft, fourier_filter_br):
    """Fold twiddles+filter into the dense operator W[o, i] (512x512 f64)."""
    tw_fft = np.asarray(twiddle_fft, dtype=np.float64)
    tw_ifft = np.asarray(twiddle_ifft, dtype=np.float64)
    filt = np.asarray(fourier_filter_br, dtype=np.float64)
    tf = tw_fft[0, ..., 0] + 1j * tw_fft[0, ..., 1]
    ti = tw_ifft[0, ..., 0] + 1j * tw_ifft[0, ..., 1]
    X = np.eye(NF, dtype=np.complex128)
    X = _butterfly_np(tf, X, increasing=False)
    X = X * filt[None, :]
    X = _butterfly_np(ti, X, increasing=True)
    return np.real(X[:S, :S]).T.copy()


def _band_stationaries(W):
    """lhsT operands: lhsT_a[i', o'] = g[o'-i'+64], lhsT_b = g[o'-i'-64]."""
    g = W[:, 0]  # g[|d|]; W is symmetric Toeplitz to ~3e-8
    D = np.arange(P)[None, :] - np.arange(P)[:, None]  # D[i', o'] = o' - i'
    return g[np.abs(D + H)], g[np.abs(D - H)]


def _build_nc():
    bf16 = mybir.dt.bfloat16
    f32 = mybir.dt.float32

    nc = bass.Bass()
    # Input pieces in Sync-queue (= PE consumption) order, packed for fat
    # partition lines (4 KiB lines move ~257 GB/s vs ~190 at 2 KiB):
    #   p0 = (128, 1280) [Ga | Gb | c4_t0 | c4_t1]
    #   p1 = (128, 2048) [c0|c1|c2|c3] tile0     (4 KiB lines)
    #   p2 = (128, 1024) [c0|c1] tile1, p3 = (128, 1024) [c2|c3] tile1
    p0 = nc.declare_dram_parameter("p0", [P, 2 * P + 2 * S], bf16, isOutput=False)
    p1 = nc.declare_dram_parameter("p1", [P, 4 * S], bf16, isOutput=False)
    p2 = nc.declare_dram_parameter("p2", [P, 2 * S], bf16, isOutput=False)
    p3 = nc.declare_dram_parameter("p3", [P, 2 * S], bf16, isOutput=False)
    out0 = nc.declare_dram_parameter("out0", [P, OC * S], bf16, isOutput=True)
    out1 = nc.declare_dram_parameter("out1", [P, OC * S], bf16, isOutput=True)

    with ExitStack() as ctx:
        w_sb = ctx.enter_context(
            nc.sbuf_tensor("w_sb", [P, 2 * P + 2 * S], bf16)
        )
        p1_sb = ctx.enter_context(nc.sbuf_tensor("p1_sb", [P, 4 * S], bf16))
        p2_sb = ctx.enter_context(nc.sbuf_tensor("p2_sb", [P, 2 * S], bf16))
        p3_sb = ctx.enter_context(nc.sbuf_tensor("p3_sb", [P, 2 * S], bf16))
        warm_sb = ctx.enter_context(nc.sbuf_tensor("warm_sb", [P, P + S], bf16))
        o_sb = [
            ctx.enter_context(nc.sbuf_tensor(f"o_sb{t}", [P, OC * S], bf16))
            for t in range(BC_PER_CORE)
        ]
        accs = [
            ctx.enter_context(nc.psum_tensor(f"acc{g}", [P, S], f32))
            for g in range(BC_PER_CORE * OC)
        ]
        s_i = [ctx.enter_context(nc.semaphore(f"s_i{j}")) for j in range(4)]
        s_pe = ctx.enter_context(nc.semaphore("s_pe"))
        s_cl = ctx.enter_context(nc.semaphore("s_cl"))   # DVE copies
        s_cr = ctx.enter_context(nc.semaphore("s_cr"))   # ACT copies
        s_out = ctx.enter_context(nc.semaphore("s_out"))
        block = ctx.enter_context(nc.Block())

        ga = w_sb[:, 0:P]
        gb = w_sb[:, P:2 * P]
        cs = [
            [
                p1_sb[:, bass.ts(j, S)] for j in range(4)
            ] + [w_sb[:, 2 * P:2 * P + S]],
            [
                p2_sb[:, 0:S], p2_sb[:, S:2 * S],
                p3_sb[:, 0:S], p3_sb[:, S:2 * S],
                w_sb[:, 2 * P + S:],
            ],
        ]

        @block.sync
        def _(sync):
            sync.dma_start(w_sb[:], p0[:]).then_inc(s_i[0], 16)
            sync.dma_start(p1_sb[:], p1[:]).then_inc(s_i[1], 16)
            sync.dma_start(p2_sb[:], p2[:]).then_inc(s_i[2], 16)
            sync.dma_start(p3_sb[:], p3[:]).then_inc(s_i[3], 16)
            # tile1 [o0|o1] after copies g4 (DVE #4) and g5 (ACT #2)
            sync.wait_ge(s_cl, 4)
            sync.wait_ge(s_cr, 2)
            sync.dma_start(out1[:, :2 * S], o_sb[1][:, :2 * S]).then_inc(s_out, 16)
            sync.wait_ge(s_out, 3 * 16)

        @block.tensor
        def _(tensor):
            # Fine-grained warm-ups (256 moving rows, ~200 ns each) so the
            # warm stream can end within one matmul of the first piece's
            # arrival: any longer PE idle gap resets the HAM ramp credit.
            for _ in range(N_WARM):
                nc.tensor.matmul(
                    accs[-1][:, :2 * P], warm_sb[:, :P], warm_sb[:, P:P + 2 * P],
                    start=True, stop=True,
                )
            tensor.wait_ge(s_i[0], 16)
            for t in range(BC_PER_CORE):
                a = OC * t
                c = cs[t]

                def mm(g, w_ap, c_ap, start, stop):
                    m = nc.tensor.matmul(
                        accs[g][:], w_ap, c_ap, start=start, stop=stop
                    )
                    if stop:
                        m.then_inc(s_pe, 1)

                if t == 0:
                    tensor.wait_ge(s_i[1], 16)
                    mm(a + 0, ga, c[0], True, False)
                    mm(a + 1, ga, c[1], True, False)
                    mm(a + 0, gb, c[1], False, True)
                else:
                    tensor.wait_ge(s_i[2], 16)
                    mm(a + 0, ga, c[0], True, False)
                    mm(a + 1, ga, c[1], True, False)
                    mm(a + 0, gb, c[1], False, True)
                    tensor.wait_ge(s_i[3], 16)
                mm(a + 2, ga, c[2], True, False)
                mm(a + 1, gb, c[2], False, True)
                mm(a + 3, ga, c[3], True, False)
                mm(a + 2, gb, c[3], False, True)
                mm(a + 3, gb, c[4], False, True)

        @block.vector
        def _(vector):
            # DVE drains groups 0,1,2 (tile0) and 4,6,7 (tile1); then_inc
            # rides the copy itself (v3.1-proven safe for DVE).
            for g, thr in ((0, 1), (1, 2), (2, 3), (4, 5), (6, 7), (7, 8)):
                t, o = divmod(g, OC)
                vector.wait_ge(s_pe, thr)
                nc.vector.tensor_copy(
                    o_sb[t][:, bass.ts(o, S)], accs[g][:]
                ).then_inc(s_cl, 1)

        @block.scalar
        def _(scalar):
            # ACT drains the two mid-stream groups 3 and 5 (signalled via an
            # explicit pipeline drain for DMA-read visibility) and issues the
            # out0 / out1-second-half DMAs on its queue.
            scalar.wait_ge(s_pe, 4)
            nc.scalar.copy(o_sb[0][:, bass.ts(3, S)], accs[3][:])
            scalar.drain().then_inc(s_cr, 1)
            scalar.wait_ge(s_cl, 3)
            scalar.dma_start(out0[:], o_sb[0][:]).then_inc(s_out, 16)
            scalar.wait_ge(s_pe, 6)
            nc.scalar.copy(o_sb[1][:, bass.ts(1, S)], accs[5][:])
            scalar.drain().then_inc(s_cr, 1)
            # tile1 [o2|o3] after DVE copies g6 (#5) and g7 (#6)
            scalar.wait_ge(s_cl, 6)
            scalar.dma_start(
                out1[:, 2 * S:], o_sb[1][:, 2 * S:]
            ).then_inc(s_out, 16)

    return nc


def kernel(x, twiddle_fft, twiddle_ifft, fourier_filter_br):
    global last_exec_time_ns, last_results
    import ml_dtypes

    bf16 = ml_dtypes.bfloat16
    x = np.asarray(x, dtype=np.float32)
    b, c, s_len, a = x.shape
    assert (b, c, s_len, a) == (8, 2, S, S)

    W = _compose_w(twiddle_fft, twiddle_ifft, fourier_filter_br)
    la, lb = _band_stationaries(W)
    w_piece = np.ascontiguousarray(
        np.concatenate([la, lb], axis=1).astype(bf16)
    )
    x16 = x.reshape(b * c, S, S)
    zpad = np.zeros((H, S), dtype=bf16)

    in_maps = []
    for core in range(N_CORES):
        cks = []
        for t in range(BC_PER_CORE):
            xb = x16[BC_PER_CORE * core + t].astype(bf16)
            cks.append(
                [
                    np.concatenate([zpad, xb[0:H]], axis=0),
                    xb[H:H + P],
                    xb[H + P:H + 2 * P],
                    xb[H + 2 * P:H + 3 * P],
                    np.concatenate([xb[H + 3 * P:], zpad], axis=0),
                ]
            )
        cat = lambda parts: np.ascontiguousarray(np.concatenate(parts, axis=1))
        in_maps.append(
            {
                "p0": cat([w_piece, cks[0][4], cks[1][4]]),
                "p1": cat(cks[0][0:4]),
                "p2": cat(cks[1][0:2]),
                "p3": cat(cks[1][2:4]),
            }
        )
    nc = _build_nc()
    trace = os.environ.get("BUTTERFLY_TRACE") == "1"
    res = run_bass_kernel_spmd(nc, in_maps, core_ids=list(range(N_CORES)), trace=trace)
    last_exec_time_ns = res.exec_time_ns
    last_results = res

    # outN[p, 512*o + a] = proj row 128*o + p of tile 2*core + N.
    q = np.empty((b * c, S, S), dtype=np.float32)
    for k in range(N_CORES):
        for t, name in enumerate(("out0", "out1")):
            y = np.asarray(res.results[k][name]).reshape(P, OC, S)
            q[BC_PER_CORE * k + t] = (
                y.transpose(1, 0, 2).reshape(S, S).astype(np.float32)
            )
    # q[bc, o, a] = proj.T[o, bc*512 + a]; reference output is
    # proj.T.reshape(b, c, s, a) — a pure reinterpret of the (512, 8192) buffer.
    out = q.transpose(1, 0, 2).reshape(S, b * c * a).reshape(b, c, s_len, a)
    return np.ascontiguousarray(out).astype(np.float32)


# revision 30
# speedup vs baseline: 1.2038x; 1.0227x over previous
"""Trainium2 Bass kernel for nn_ButterflyFilter.

The reference chain (pad -> butterfly FFT -> ramp filter in bit-reversed
order -> butterfly IFFT -> Re[:512]) is linear in x, so it is one real
512x512 operator W = Re(A)[:512, :512] with A circulant. W is an exactly
symmetric Toeplitz matrix W[o, i] = g[o - i] with g the FBP ramp kernel
(g[0] = 1/2, g[odd d] = -2/(pi d)^2, g[even d] = 0), which decays like
1/d^2: a 64-wide staircase band changes the result by ~1.6e-4 relative;
with bf16 operands and output store the total is ~2.6e-3 (measured),
7x under the 2e-2 gate.

Banded + Toeplitz => each 128-row output chunk needs TWO input chunks on
a 64-shifted grid:
  out[128o : 128o+128] = Ga @ c_o + Gb @ c_{o+1},
  c_j = x rows [128j - 64, 128j + 64)   (zero-padded at the ends)
with the same two 128x128 stationaries for every o: 8 matmuls per
(b, c) tile, 16 per core (2 tiles/core, 8 cores), 64 KiB of operator.

Schedule facts this implementation is built around (from NTFF traces):
  - The DMA fabric is ~270 GB/s per core AGGREGATE across queues, so
    queues are specialized: Sync carries the 5 input pieces in exact PE
    consumption order; Scalar's queue carries outputs. Competing input
    streams starve the PE mid-stream.
  - A dma_start costs ~0.6-0.7 us of descriptor-gen on the issuing
    engine: inputs are fused into 5 pieces (64K + 256K/384K per tile)
    with 1-3 KiB partition lines.
  - Concurrently in-flight DMAs must not share a semaphore (their 16
    completion increments interleave out of order): one per piece.
  - PSUM->SBUF bf16 drains: DVE tensor_copy signals with then_inc
    directly (proven safe); ACT activation-copies are kept OFF the
    critical tail and signal via an explicit pipeline drain.
  - ~6 warm-up matmuls on garbage SBUF bridge program start to the
    first piece's arrival so the HAM clock ramp (1.2 -> 2.4 GHz after
    ~3.4 us of sustained PE activity) completes before the real stream;
    any PE idle gap resets the ramp credit.
"""

import os
import sys
import types
from contextlib import ExitStack

import numpy as np

import concourse.bass as bass
import concourse.mybir as mybir
from concourse.bass_utils import run_bass_kernel_spmd


def _ensure_axon_hooks():
    # concourse.bass_utils imports antenv.axon_hooks on the trace path; some
    # images lack that module. Provide a no-op holder so a BASS_TRACE env set
    # by the caller can't crash the run.
    try:
        import antenv.axon_hooks  # noqa: F401
    except Exception:
        m = types.ModuleType("antenv.axon_hooks")
        m._h = None
        m.set_axon_ntff_profile_hook = lambda h: setattr(m, "_h", h)
        m.get_axon_ntff_profile_hook = lambda: m._h
        sys.modules["antenv.axon_hooks"] = m


_ensure_axon_hooks()

N_CORES = 8
S = 512          # row length and angle count (moving dim)
NF = 1024        # padded length inside the reference
P = 128
H = 64           # chunk-grid shift
OC = 4           # output row chunks per tile
BC_PER_CORE = 2
N_WARM = int(os.environ.get("BUTTERFLY_NWARM", "20"))

last_exec_time_ns = None
last_results = None


def _butterfly_np(tw, x, increasing):
    B, n = x.shape
    m = tw.shape[0]
    order = range(m) if increasing else range(m - 1, -1, -1)
    for idx in order:
        s = 1 << idx
        t = tw[idx].reshape(n // (2 * s), s, 2, 2)
        xr = x.reshape(B, n // (2 * s), 2, s)
        x = np.einsum('gjik,bgkj->bgij', t, xr).reshape(B, n)
    return x


def _compose_w(twiddle_fft, twiddle_ifft, fourier_filter_br):
    """Fold twiddles+filter into the dense operator W[o, i] (512x512 f64)."""
    tw_fft = np.asarray(twiddle_fft, dtype=np.float64)
    tw_ifft = np.asarray(twiddle_ifft, dtype=np.float64)
    filt = np.asarray(fourier_filter_br, dtype=np.float64)
    tf = tw_fft[0, ..., 0] + 1j * tw_fft[0, ..., 1]
    ti = tw_ifft[0, ..., 0] + 1j * tw_ifft[0, ..., 1]
    X = np.eye(NF, dtype=np.complex128)
    X = _butterfly_np(tf, X, increasing=False)
    X = X * filt[None, :]
    X = _butterfly_np(ti, X, increasing=True)
    return np.real(X[:S, :S]).T.copy()


def _band_stationaries(W):
    """lhsT operands: lhsT_a[i', o'] = g[o'-i'+64], lhsT_b = g[o'-i'-64]."""
    g = W[:, 0]  # g[|d|]; W is symmetric Toeplitz to ~3e-8
    D = np.arange(P)[None, :] - np.arange(P)[:, None]  # D[i', o'] = o' - i'
    return g[np.abs(D + H)], g[np.abs(D - H)]


def _build_nc():
    bf16 = mybir.dt.bfloat16
    f32 = mybir.dt.float32

    nc = bass.Bass()
    # Input pieces in Sync-queue (= PE consumption) order, packed for fat
    # partition lines (4 KiB lines move ~257 GB/s vs ~190 at 2 KiB). The
    # operator piece goes alone first (64 KiB) so the stream starts early;
    # the c4 chunks ride a later piece (first needed by matmul #8).
    #   p0 = (128, 256)  [Ga | Gb]
    #   p1 = (128, 2048) [c0|c1|c2|c3] tile0     (4 KiB lines)
    #   pc4 = (128, 1024) [c4_t0 | c4_t1]
    #   p2 = (128, 1024) [c0|c1] tile1, p3 = (128, 1024) [c2|c3] tile1
    p0 = nc.declare_dram_parameter("p0", [P, 2 * P], bf16, isOutput=False)
    p1 = nc.declare_dram_parameter("p1", [P, 4 * S], bf16, isOutput=False)
    pc4 = nc.declare_dram_parameter("pc4", [P, 2 * S], bf16, isOutput=False)
    p2 = nc.declare_dram_parameter("p2", [P, 2 * S], bf16, isOutput=False)
    p3 = nc.declare_dram_parameter("p3", [P, 2 * S], bf16, isOutput=False)
    out0 = nc.declare_dram_parameter("out0", [P, OC * S], bf16, isOutput=True)
    out1 = nc.declare_dram_parameter("out1", [P, OC * S], bf16, isOutput=True)

    with ExitStack() as ctx:
        w_sb = ctx.enter_context(nc.sbuf_tensor("w_sb", [P, 2 * P], bf16))
        pc4_sb = ctx.enter_context(nc.sbuf_tensor("pc4_sb", [P, 2 * S], bf16))
        p1_sb = ctx.enter_context(nc.sbuf_tensor("p1_sb", [P, 4 * S], bf16))
        p2_sb = ctx.enter_context(nc.sbuf_tensor("p2_sb", [P, 2 * S], bf16))
        p3_sb = ctx.enter_context(nc.sbuf_tensor("p3_sb", [P, 2 * S], bf16))
        warm_sb = ctx.enter_context(nc.sbuf_tensor("warm_sb", [P, P + S], bf16))
        o_sb = [
            ctx.enter_context(nc.sbuf_tensor(f"o_sb{t}", [P, OC * S], bf16))
            for t in range(BC_PER_CORE)
        ]
        accs = [
            ctx.enter_context(nc.psum_tensor(f"acc{g}", [P, S], f32))
            for g in range(BC_PER_CORE * OC)
        ]
        s_i = [ctx.enter_context(nc.semaphore(f"s_i{j}")) for j in range(5)]
        s_pe = ctx.enter_context(nc.semaphore("s_pe"))
        s_cl = ctx.enter_context(nc.semaphore("s_cl"))   # DVE copies
        s_cr = ctx.enter_context(nc.semaphore("s_cr"))   # ACT copies
        s_out = ctx.enter_context(nc.semaphore("s_out"))
        block = ctx.enter_context(nc.Block())

        ga = w_sb[:, 0:P]
        gb = w_sb[:, P:2 * P]
        cs = [
            [
                p1_sb[:, bass.ts(j, S)] for j in range(4)
            ] + [pc4_sb[:, 0:S]],
            [
                p2_sb[:, 0:S], p2_sb[:, S:2 * S],
                p3_sb[:, 0:S], p3_sb[:, S:2 * S],
                pc4_sb[:, S:2 * S],
            ],
        ]

        @block.sync
        def _(sync):
            sync.dma_start(w_sb[:], p0[:]).then_inc(s_i[0], 16)
            sync.dma_start(p1_sb[:], p1[:]).then_inc(s_i[1], 16)
            sync.dma_start(pc4_sb[:], pc4[:]).then_inc(s_i[4], 16)
            sync.dma_start(p2_sb[:], p2[:]).then_inc(s_i[2], 16)
            sync.dma_start(p3_sb[:], p3[:]).then_inc(s_i[3], 16)
            # tile1 [o0|o1] after DVE copies g4 (#2) and g5 (#3)
            sync.wait_ge(s_cl, 3)
            sync.dma_start(out1[:, :2 * S], o_sb[1][:, :2 * S]).then_inc(s_out, 16)
            # tile1 [o2|o3] after DVE copies g6 (#4) and g7 (#5)
            sync.wait_ge(s_cl, 5)
            sync.dma_start(out1[:, 2 * S:], o_sb[1][:, 2 * S:]).then_inc(s_out, 16)
            sync.wait_ge(s_out, 3 * 16)

        @block.tensor
        def _(tensor):
            # Fine-grained warm-ups (256 moving rows, ~200 ns each) so the
            # warm stream can end within one matmul of the first piece's
            # arrival: any longer PE idle gap resets the HAM ramp credit.
            for _ in range(N_WARM):
                nc.tensor.matmul(
                    accs[-1][:, :2 * P], warm_sb[:, :P], warm_sb[:, P:P + 2 * P],
                    start=True, stop=True,
                )
            tensor.wait_ge(s_i[0], 16)
            for t in range(BC_PER_CORE):
                a = OC * t
                c = cs[t]

                def mm(g, w_ap, c_ap, start, stop):
                    m = nc.tensor.matmul(
                        accs[g][:], w_ap, c_ap, start=start, stop=stop
                    )
                    if stop:
                        m.then_inc(s_pe, 1)

                # Ga/Gb alternated so a group closes every 2nd matmul — the
                # copy engines start draining as early as possible.
                if t == 0:
                    tensor.wait_ge(s_i[1], 16)
                    mm(a + 0, ga, c[0], True, False)
                    mm(a + 0, gb, c[1], False, True)
                    mm(a + 1, ga, c[1], True, False)
                    mm(a + 1, gb, c[2], False, True)
                    mm(a + 2, ga, c[2], True, False)
                    mm(a + 2, gb, c[3], False, True)
                    mm(a + 3, ga, c[3], True, False)
                    tensor.wait_ge(s_i[4], 16)
                    mm(a + 3, gb, c[4], False, True)
                else:
                    tensor.wait_ge(s_i[2], 16)
                    mm(a + 0, ga, c[0], True, False)
                    mm(a + 1, ga, c[1], True, False)
                    mm(a + 0, gb, c[1], False, True)
                    tensor.wait_ge(s_i[3], 16)
                    mm(a + 1, gb, c[2], False, True)
                    mm(a + 2, ga, c[2], True, False)
                    mm(a + 2, gb, c[3], False, True)
                    mm(a + 3, ga, c[3], True, False)
                    mm(a + 3, gb, c[4], False, True)

        @block.vector
        def _(vector):
            # DVE drains g0 and every tail-critical group (g4..g7); then_inc
            # rides the copy itself (v3.1-proven safe for DVE).
            for g in (0, 4, 5, 6, 7):
                t, o = divmod(g, OC)
                vector.wait_ge(s_pe, g + 1)
                nc.vector.tensor_copy(
                    o_sb[t][:, bass.ts(o, S)], accs[g][:]
                ).then_inc(s_cl, 1)

        @block.scalar
        def _(scalar):
            # ACT drains the early groups g1..g3 back-to-back, signals once
            # via a single pipeline drain, then issues tile0's output.
            for g in (1, 2, 3):
                scalar.wait_ge(s_pe, g + 1)
                nc.scalar.copy(o_sb[0][:, bass.ts(g, S)], accs[g][:])
            scalar.drain().then_inc(s_cr, 3)
            scalar.wait_ge(s_cl, 1)
            scalar.dma_start(out0[:], o_sb[0][:]).then_inc(s_out, 16)

    return nc


def kernel(x, twiddle_fft, twiddle_ifft, fourier_filter_br):
    global last_exec_time_ns, last_results
    import ml_dtypes

    bf16 = ml_dtypes.bfloat16
    x = np.asarray(x, dtype=np.float32)
    b, c, s_len, a = x.shape
    assert (b, c, s_len, a) == (8, 2, S, S)

    W = _compose_w(twiddle_fft, twiddle_ifft, fourier_filter_br)
    la, lb = _band_stationaries(W)
    w_piece = np.ascontiguousarray(
        np.concatenate([la, lb], axis=1).astype(bf16)
    )
    x16 = x.reshape(b * c, S, S)
    zpad = np.zeros((H, S), dtype=bf16)

    in_maps = []
    for core in range(N_CORES):
        cks = []
        for t in range(BC_PER_CORE):
            xb = x16[BC_PER_CORE * core + t].astype(bf16)
            cks.append(
                [
                    np.concatenate([zpad, xb[0:H]], axis=0),
                    xb[H:H + P],
                    xb[H + P:H + 2 * P],
                    xb[H + 2 * P:H + 3 * P],
                    np.concatenate([xb[H + 3 * P:], zpad], axis=0),
                ]
            )
        cat = lambda parts: np.ascontiguousarray(np.concatenate(parts, axis=1))
        in_maps.append(
            {
                "p0": np.ascontiguousarray(w_piece),
                "p1": cat(cks[0][0:4]),
                "pc4": cat([cks[0][4], cks[1][4]]),
                "p2": cat(cks[1][0:2]),
                "p3": cat(cks[1][2:4]),
            }
        )
    nc = _build_nc()
    trace = os.environ.get("BUTTERFLY_TRACE") == "1"
    res = run_bass_kernel_spmd(nc, in_maps, core_ids=list(range(N_CORES)), trace=trace)
    last_exec_time_ns = res.exec_time_ns
    last_results = res

    # outN[p, 512*o + a] = proj row 128*o + p of tile 2*core + N.
    q = np.empty((b * c, S, S), dtype=np.float32)
    for k in range(N_CORES):
        for t, name in enumerate(("out0", "out1")):
            y = np.asarray(res.results[k][name]).reshape(P, OC, S)
            q[BC_PER_CORE * k + t] = (
                y.transpose(1, 0, 2).reshape(S, S).astype(np.float32)
            )
    # q[bc, o, a] = proj.T[o, bc*512 + a]; reference output is
    # proj.T.reshape(b, c, s, a) — a pure reinterpret of the (512, 8192) buffer.
    out = q.transpose(1, 0, 2).reshape(S, b * c * a).reshape(b, c, s_len, a)
    return np.ascontiguousarray(out).astype(np.float32)
